# revision 18
# baseline (speedup 1.0000x reference)
"""CRF Viterbi decode (nn_CrfDecodeLayer) Trainium2 Bass kernel.

Problem: B=256, T=512, K=256 tags. Forward max-plus scan over T with
transition matrix trans[K,K], then backtrack to recover argmax tag path.
Output: tags [B, 514] int32 (padded to max_sequence_length + 2).

Sharding: data-parallel over batch: B=256 -> 8 cores x 32.

Per-core algorithm (B_loc=32, exact fp32, bit-identical to the jax ref):
  forward t=1..T-1 via ONE custom DVE op (MAXPLUS_ACC_ANT):
    body = Src0 + Src1 + C0, accum = MAX:
      accum_out[p] = max_i(tr[i, j(p,m)] + s[b(p), i] + em[b(p), t, j(p,m)])
    64 instrs/step, partition p=(jq*32+b), instr m covers j = 64*jq + m.
    accum_out written into SNEW [128, 64] = the folded lattice layout
    lat[t][(jq,b), m] = s_t[b, 64*jq+m] (em included; exact: adding a
    per-j constant before an exact max-fold == adding it after).
  state unfold: 4 PE matmuls (stationary rep32 = eye32 tiled 4x) move
    SNEW group-rows to all partitions -> pre PSUM [128,256] = s_t[b, i]
    replicated over the 4 partition groups; one DVE copy -> SBUF s_rep
    (the Src1 stream of step t+1).
  backtrack (recompute argmax instead of storing backpointers):
    tag_T-1 = argmax_j s_T-1[b,j]
    tag_t = argmax_i(s_t[b,i] + trans[i, tag_{t+1}])   (first-index ties)
    trans^T row gather via gpsimd indirect DMA fused with +s_t
    (compute_op=add); two batch-half chains interleaved.
"""

import numpy as np

B, T, K = 256, 512, 256
NCORES = 8
BLOC = B // NCORES  # 32
OUT_T = T + 2  # 514
NCHAIN = 1  # backtrack chains (batch split)

_MAXPLUS = None


def _register_maxplus():
    """Register the custom DVE op (idempotent): accum_out = max-fold of
    (Src0 + Src1 + C0)."""
    global _MAXPLUS
    if _MAXPLUS is not None:
        return _MAXPLUS
    import concourse.dve_ops as dve_ops
    from concourse.dve_spec import Spec, Src0, Src1, C0, maxx, lower, _has_src1
    from concourse.dve_uop import DveOpSpec

    name = "MAXPLUS_ACC_ANT"
    for o in dve_ops.OPS:
        if o.name == name:
            _MAXPLUS = o
            return o
    spec = Spec(body=Src0 + Src1 + C0, accum=maxx)
    row = dve_ops._CUSTOM_DVE_ROW_BASE + len(dve_ops.OPS)
    assert row < 0x20
    shas = {}
    for ver in ("v3", "v4"):
        s = DveOpSpec(name=name, opcode=row, uops=lower(spec, ver=ver),
                      rd1_en=_has_src1(spec))
        shas[ver] = s.sha(ver)
    op = dve_ops.DveOp(name, spec, subdim=False, uops_sha=shas)
    dve_ops.OPS.append(op)
    dve_ops._SUB_OPCODE_FOR_NAME[name] = row
    _MAXPLUS = op
    return op


def build_program(t_steps: int = T):
    """Build the SPMD Bass program (same program for all 8 cores)."""
    from contextlib import ExitStack

    import concourse.bass as bass
    import concourse.tile as tile
    from concourse import bacc, mybir

    MAXPLUS = _register_maxplus()

    FP32 = mybir.dt.float32
    INT32 = mybir.dt.int32
    A = mybir.AluOpType

    nc = bacc.Bacc("TRN2", target_bir_lowering=False, num_devices=NCORES)

    # ---- DRAM I/O ----
    # em_f[t, jq*32+b, m] = emissions[b, t, 64*jq+m]  (folded layout)
    em_f_d = nc.dram_tensor("em_f", [t_steps, 128, 64], FP32, kind="ExternalInput")
    # trs[m, jq*32+b, i] = trans[i, 64*jq+m]  (stream layout, b-replicated)
    trs_d = nc.dram_tensor("trs", [64, 128, K], FP32, kind="ExternalInput")
    transT_d = nc.dram_tensor("transT", [K, K], FP32, kind="ExternalInput")
    sel4_d = nc.dram_tensor("sel4", [4, 128, 128], FP32, kind="ExternalInput")
    tags_d = nc.dram_tensor("tags", [BLOC, OUT_T], INT32, kind="ExternalOutput")
    # folded lattice: lat[t, jq*32+b, m] = s_t[b, 64*jq+m]
    lat_d = nc.dram_tensor("lat", [t_steps, 128, 64], FP32)

    with tile.TileContext(nc) as tc:
        with ExitStack() as ctx:
            static_pool = ctx.enter_context(tc.tile_pool(name="static", bufs=1))
            srep_pool = ctx.enter_context(tc.tile_pool(name="srep", bufs=3))
            pre_pool = ctx.enter_context(tc.tile_pool(name="pre", bufs=4, space="PSUM"))
            snew_pool = ctx.enter_context(tc.tile_pool(name="snew", bufs=3))
            scr_pool = ctx.enter_context(tc.tile_pool(name="scr", bufs=2))
            em_pool = ctx.enter_context(tc.tile_pool(name="em", bufs=6))
            bt_pool = ctx.enter_context(tc.tile_pool(name="bt", bufs=12))
            sm_pool = ctx.enter_context(tc.tile_pool(name="sm", bufs=4))

            # ---- static loads ----
            trs = static_pool.tile([128, 64, K], FP32)
            nc.sync.dma_start(trs[:], trs_d.ap().transpose([1, 0, 2]))
            sel4 = static_pool.tile([128, 4, 128], FP32)
            nc.sync.dma_start(sel4[:], sel4_d.ap().transpose([1, 0, 2]))
            CHB = [(BLOC * c // NCHAIN, BLOC * (c + 1) // NCHAIN) for c in range(NCHAIN)]
            tags_fc = [
                static_pool.tile([hi - lo, T], FP32, name=f"tagsf{c}", tag=f"tagsf{c}")
                for c, (lo, hi) in enumerate(CHB)
            ]

            em_tiles = {}

            def em_load(t):
                if t >= t_steps:
                    return
                em_t = em_pool.tile([128, 64], FP32)
                nc.scalar.dma_start(em_t[:], em_f_d.ap()[t])
                em_tiles[t] = em_t

            def unfold_mm(pre_half, half_tile):
                """pre_half[:, jq*32+mm] = snew_half[jq*32 + p%32, mm] via 4
                PE matmuls on a [128, 32] snew-half tile; own PSUM tile per
                half so the two halves never serialize on tile deps."""
                for jq in range(4):
                    nc.tensor.matmul(
                        pre_half[:, jq * 32 : (jq + 1) * 32],
                        sel4[:, jq, :],
                        half_tile[:],
                        start=True,
                        stop=True,
                    )

            def unfold_copy(pre_half, s_rep, mlo, eng):
                pv = pre_half[:].rearrange("p (jq mm) -> p jq mm", jq=4)
                sv = s_rep[:].rearrange("p (jq m) -> p jq m", jq=4)[:, :, mlo : mlo + 32]
                if eng == "act":
                    nc.scalar.copy(sv, pv)
                else:
                    nc.vector.tensor_copy(out=sv, in_=pv)

            def unfold(snew_a, snew_b):
                pre_a = pre_pool.tile([128, 128], FP32, name="prea", tag="prea")
                pre_b = pre_pool.tile([128, 128], FP32, name="preb", tag="preb")
                s_rep = srep_pool.tile([128, K], FP32)
                unfold_mm(pre_a, snew_a)
                unfold_copy(pre_a, s_rep, 0, "act")
                unfold_mm(pre_b, snew_b)
                unfold_copy(pre_b, s_rep, 32, "vec")
                return s_rep

            # ---- t = 0: s_0 = em_0 ----
            em0a = snew_pool.tile([128, 32], FP32, name="em0a", tag="sna")
            em0b = snew_pool.tile([128, 32], FP32, name="em0b", tag="snb")
            nc.sync.dma_start(em0a[:], em_f_d.ap()[0][:, 0:32])
            nc.sync.dma_start(em0b[:], em_f_d.ap()[0][:, 32:64])
            nc.gpsimd.dma_start(lat_d.ap()[0], em_f_d.ap()[0])
            s_rep = unfold(em0a, em0b)
            for t in (1, 2, 3):
                em_load(t)

            # ---- forward scan ----
            for t in range(1, t_steps):
                em_t = em_tiles.pop(t)
                snew_a = snew_pool.tile([128, 32], FP32, name="snewa", tag="sna")
                snew_b = snew_pool.tile([128, 32], FP32, name="snewb", tag="snb")
                last = t == t_steps - 1
                if not last:
                    pre_a = pre_pool.tile([128, 128], FP32, name="prea", tag="prea")
                    pre_b = pre_pool.tile([128, 128], FP32, name="preb", tag="preb")
                    s_rep_next = srep_pool.tile([128, K], FP32)
                em_load(t + 3)
                for m in range(64):
                    scr = scr_pool.tile([128, K], FP32, name=f"scr{m % 2}", tag=f"scr{m % 2}")
                    sn = snew_a if m < 32 else snew_b
                    nc.vector._custom_dve(
                        MAXPLUS,
                        out=scr[:],
                        in0=trs[:, m, :],
                        in1=s_rep[:],
                        s0=em_t[:, m : m + 1],
                        accum_out=sn[:, m % 32 : m % 32 + 1],
                    )
                    if m == 31:
                        # first-half unfold + lat store overlap the second
                        # half of the MAXPLUS block (snew_a is complete)
                        nc.gpsimd.dma_start(lat_d.ap()[t][:, 0:32], snew_a[:])
                        if not last:
                            unfold_mm(pre_a, snew_a)
                    elif m == 47 and not last:
                        unfold_copy(pre_a, s_rep_next, 0, "act")
                nc.gpsimd.dma_start(lat_d.ap()[t][:, 32:64], snew_b[:])
                if not last:
                    # interleave each strip copy right after its matmul so
                    # the copies pipeline against the remaining matmuls
                    for jq in range(4):
                        nc.tensor.matmul(
                            pre_b[:, jq * 32 : (jq + 1) * 32],
                            sel4[:, jq, :],
                            snew_b[:],
                            start=True,
                            stop=True,
                        )
                        nc.vector.tensor_copy(
                            out=s_rep_next[:, jq * 64 + 32 : jq * 64 + 64],
                            in_=pre_b[:, jq * 32 : (jq + 1) * 32],
                        )
                    s_rep = s_rep_next

            # ---- backtrack: NCHAIN independent batch-slice chains ----
            def lat_rows(t, lo, hi):
                # [hi-lo, K] row view of folded lat[t]:
                # dst[b, 64*jq+m] = lat[t, jq*32 + lo + b, m]
                return lat_d.ap()[t].rearrange("(jq bb) m -> bb jq m", jq=4)[lo:hi]

            def argmax_step(val, t_col, c):
                # top-8 values then first-occurrence index match: argmax with
                # first-index tie semantics in 2 DVE ops, index out as uint32
                nb = CHB[c][1] - CHB[c][0]
                m8 = sm_pool.tile([nb, 8], FP32, name=f"am{c}", tag=f"m{c}")
                nc.vector.max(m8[:], val[:])
                idx8 = sm_pool.tile(
                    [nb, 8], mybir.dt.uint32, name=f"aidx{c}", tag=f"idx{c}"
                )
                nc.vector.max_index(idx8[:], m8[:], val[:])
                nc.scalar.copy(tags_fc[c][:, t_col : t_col + 1], idx8[:, 0:1])
                return idx8

            idxs = [None] * NCHAIN
            for c, (lo, hi) in enumerate(CHB):
                sv = bt_pool.tile([hi - lo, K], FP32, name=f"sv{c}", tag=f"sv{c}")
                nc.sync.dma_start(sv[:], lat_rows(t_steps - 1, lo, hi))
                idxs[c] = argmax_step(sv, t_steps - 1, c)

            for t in range(t_steps - 2, -1, -1):
                svs = []
                for c, (lo, hi) in enumerate(CHB):
                    sv = bt_pool.tile([hi - lo, K], FP32, name=f"svl{c}", tag=f"sv{c}")
                    eng = nc.sync if c % 2 == 0 else nc.scalar
                    eng.dma_start(sv[:], lat_rows(t, lo, hi))
                    nc.gpsimd.indirect_dma_start(
                        out=sv[:],
                        out_offset=None,
                        in_=transT_d.ap(),
                        in_offset=bass.IndirectOffsetOnAxis(ap=idxs[c][:, :1], axis=0),
                        compute_op=A.add,
                    )
                    svs.append(sv)
                for c in range(NCHAIN):
                    idxs[c] = argmax_step(svs[c], t, c)

            # ---- output assembly (per chain; DMA merges partition offsets) ----
            for c, (lo, hi) in enumerate(CHB):
                tags_i = static_pool.tile(
                    [hi - lo, OUT_T], INT32, name=f"tagsi{c}", tag=f"tagsi{c}"
                )
                nc.vector.memset(tags_i[:], 0)
                nc.vector.tensor_scalar(
                    tags_i[:, 0:t_steps], tags_fc[c][:, 0:t_steps], 0.0, None, op0=A.add
                )
                nc.sync.dma_start(tags_d.ap()[lo:hi, :], tags_i[:])

    nc.compile()
    return nc


def _prep_inputs(emissions, transitions, t_steps: int = T):
    """Host-side layout prep. Returns per-core list of input dicts."""
    emissions = np.ascontiguousarray(emissions[:, :t_steps, :], dtype=np.float32)
    transitions = np.ascontiguousarray(transitions, dtype=np.float32)

    # trs[m, jq*32+b, i] = trans[i, 64*jq+m] = transT[64*jq+m, i]
    trT = transitions.T.reshape(4, 64, K).transpose(1, 0, 2)  # [64 m, 4 jq, K i]
    trs = np.broadcast_to(trT[:, :, None, :], (64, 4, BLOC, K)).reshape(64, 128, K)
    trs = np.ascontiguousarray(trs)
    transT = np.ascontiguousarray(transitions.T)
    # sel4[jq][k, p] = 1 iff k == jq*32 + (p % 32) (unfold stationaries)
    kk = np.arange(128)[:, None]
    pp = np.arange(128)[None, :]
    sel4 = np.stack([(kk == jq * BLOC + (pp % BLOC)).astype(np.float32)
                     for jq in range(4)])
    sel4 = np.ascontiguousarray(sel4)

    in_maps = []
    for c in range(NCORES):
        em_c = emissions[c * BLOC : (c + 1) * BLOC]  # [32, t, K]
        # em_f[t, jq*32+b, m] = em_c[b, t, 64*jq+m]
        em_f = np.ascontiguousarray(
            em_c.reshape(BLOC, t_steps, 4, 64)
            .transpose(1, 2, 0, 3)
            .reshape(t_steps, 128, 64)
        )
        in_maps.append(
            {
                "em_f": em_f,
                "trs": trs,
                "transT": transT,
                "sel4": sel4,
            }
        )
    return in_maps


def kernel(emissions, transitions, mask, max_sequence_length):
    from concourse.bass_utils import run_bass_kernel_spmd

    emissions = np.asarray(emissions)
    transitions = np.asarray(transitions)
    mask = np.asarray(mask)

    nc = build_program(T)
    in_maps = _prep_inputs(emissions, transitions, T)
    res = run_bass_kernel_spmd(nc, in_maps, list(range(NCORES)))
    tags = np.concatenate([res.results[c]["tags"] for c in range(NCORES)], axis=0)
    tags = tags.astype(np.int32)
    tags[:, :T] *= mask.astype(np.int32)
    return tags
